# revision 9
# baseline (speedup 1.0000x reference)
"""DecoderRNN (attention + LSTM cell + vocab projection) on 8 TRN2 NeuronCores.

Sharding:
  - Attention: data-parallel over batch (16 rows/core), computed in a
    transposed [feature, batch*seq] layout so every matmul contracts over
    the partition dim.
  - LSTM cell: tensor-parallel over the 4H gate dim -- core k owns hidden
    units [64k, 64k+64) of each of i/f/g/o, for the full batch.  Needs the
    full rnn_in, so the per-core context shards are all-gathered first.
  - FC (vocab): tensor-parallel over V (4000 rows/core) after all-gathering
    the h_new shards.
Outputs per core: logits slab [8,128,500], hT/cT shards [64,128]; the host
reassembles the full (prediction, h_new, c_new).
"""

import sys

if "/opt/trn_rl_repo" not in sys.path:
    sys.path.insert(0, "/opt/trn_rl_repo")

import numpy as np

V, H, B, S = 32000, 512, 128, 64
NC = 8            # cores
BL = B // NC      # 16  local batch rows (attention shard)
UL = H // NC      # 64  local hidden units (lstm shard)
VL = V // NC      # 4000 local vocab rows (fc shard)
GW = 4 * UL       # 256 local gate rows (4 gates x 64 units)
NT = 500          # fc free-dim tile (8 tiles of 500 = 4000)
HC = H // 128     # 4   128-row chunks of H
DC = 2 * H // 128  # 8  128-row chunks of 2H (rnn_in)
BS = BL * S       # 1024 local (batch, seq) positions

_CACHE = {}


def _build():
    import concourse.bass as bass
    import concourse.bacc as bacc
    import concourse.mybir as mybir
    import concourse.tile as tile
    from contextlib import ExitStack

    f32 = mybir.dt.float32
    f32r = mybir.dt.float32r
    TANH = mybir.ActivationFunctionType.Tanh
    SIG = mybir.ActivationFunctionType.Sigmoid
    EXP = mybir.ActivationFunctionType.Exp
    ADD = mybir.AluOpType.add

    nc = bacc.Bacc("TRN2", target_bir_lowering=False, debug=False, num_devices=NC)

    def din(name, shape):
        return nc.dram_tensor(name, shape, f32, kind="ExternalInput").ap()

    def dout(name, shape):
        return nc.dram_tensor(name, shape, f32, kind="ExternalOutput").ap()

    encT_d = din("encT", [H, BS])          # enc shard, transposed [d, b*s]
    WeT_d = din("WeT", [H, H])             # attn_W[:,H:].T   [d, h]
    WhT_d = din("WhT", [H, H])             # attn_W[:,:H].T   [d, h]
    h0Tl_d = din("h0Tl", [H, BL])          # local h0.T slice [d, bl]
    h0Tf_d = din("h0Tf", [H, B])           # full h0.T (replicated)
    embT_d = din("embT", [H, B])           # full embedded.T (replicated)
    attnb_d = din("attnb", [1, H])
    v4_d = din("v4", [HC, 128])            # v reshaped [4,128]
    c0T_d = din("c0T", [UL, B])            # local c0.T slice (unit shard)
    WihT_d = din("WihT", [2 * H, GW])  # W_ih rows(shard).T  [1024, 256]
    WhhT_d = din("WhhT", [H, GW])      # W_hh rows(shard).T  [512, 256]
    bih_d = din("bih", [1, GW])
    bhh_d = din("bhh", [1, GW])
    fcWT_d = din("fcWT", [V // VL, H, NT])  # [8, 512, 500] per-tile contiguous
    fcb_d = din("fcb", [1, VL])

    logits_d = dout("logits", [VL // NT, B, NT])  # [8, 128, 500]
    hT_d = dout("hT", [UL, B])
    cT_d = dout("cT", [UL, B])

    # SEL[b, bs] = 1 if bs // S == b ; SEL[BL, :] = 1  (adds attn_b row)
    sel_np = np.zeros((BL + 1, BS), np.float32)
    for b in range(BL):
        sel_np[b, b * S:(b + 1) * S] = 1.0
    sel_np[BL, :] = 1.0
    sel_d = nc.inline_tensor(sel_np, name="sel").ap()
    ones_d = nc.inline_tensor(np.ones((1, 128), np.float32), name="onesc").ap()

    with tile.TileContext(nc) as tc, ExitStack() as ctx:
        const = ctx.enter_context(tc.tile_pool(name="const", bufs=1))
        work = ctx.enter_context(tc.tile_pool(name="work", bufs=2))
        fcwp = ctx.enter_context(tc.tile_pool(name="fcw", bufs=8))
        outp = ctx.enter_context(tc.tile_pool(name="outp", bufs=3))
        psum = ctx.enter_context(tc.tile_pool(name="psum", bufs=3, space="PSUM"))
        psg = ctx.enter_context(tc.tile_pool(name="psg", bufs=2, space="PSUM"))
        dram = ctx.enter_context(tc.tile_pool(name="dram", bufs=1, space="DRAM"))

        # ---- CC-stream warmup: tiny collective, no data deps ----
        warm_in = dram.tile([1, 8], f32, tag="warm_in")
        nc.gpsimd.dma_start(warm_in[:], ones_d[:, 0:8])
        warm_out = dram.tile([NC, 8], f32, tag="warm_out")
        nc.gpsimd.collective_compute(
            "AllGather", mybir.AluOpType.bypass,
            replica_groups=[list(range(NC))],
            ins=[warm_in[:].opt()], outs=[warm_out[:].opt()])

        # ---- constant / weight loads ----
        encT = const.tile([128, HC, BS], f32, tag="encT")
        nc.sync.dma_start(encT[:].bitcast(f32r),
                          encT_d.rearrange("(c p) n -> p c n", p=128).bitcast(f32r))
        WeT = const.tile([128, HC, H], f32, tag="WeT")
        nc.sync.dma_start(WeT[:].bitcast(f32r),
                          WeT_d.rearrange("(c p) n -> p c n", p=128).bitcast(f32r))
        WhT = const.tile([128, HC, H], f32, tag="WhT")
        nc.sync.dma_start(WhT[:].bitcast(f32r),
                          WhT_d.rearrange("(c p) n -> p c n", p=128).bitcast(f32r))
        h0Tl = const.tile([128, HC, BL], f32, tag="h0Tl")
        nc.sync.dma_start(h0Tl[:].bitcast(f32r),
                          h0Tl_d.rearrange("(c p) n -> p c n", p=128).bitcast(f32r))
        h0Tf = const.tile([128, HC, B], f32, tag="h0Tf")
        nc.sync.dma_start(h0Tf[:], h0Tf_d.rearrange("(c p) n -> p c n", p=128))
        embT = const.tile([128, HC, B], f32, tag="embT")
        nc.sync.dma_start(embT[:], embT_d.rearrange("(c p) n -> p c n", p=128))
        v4 = const.tile([128, HC], f32, tag="v4")
        nc.sync.dma_start(v4[:].bitcast(f32r), v4_d.rearrange("c p -> p c").bitcast(f32r))
        selt = const.tile([BL + 1, BS], f32, tag="selt")
        nc.sync.dma_start(selt[:].bitcast(f32r), sel_d[:].bitcast(f32r))
        c0T = const.tile([UL, B], f32, tag="c0T")
        nc.sync.dma_start(c0T[:], c0T_d[:])
        WihT = const.tile([128, DC, GW], f32, tag="WihT")
        nc.sync.dma_start(WihT[:], WihT_d.rearrange("(c p) n -> p c n", p=128))
        WhhT = const.tile([128, HC, GW], f32, tag="WhhT")
        nc.sync.dma_start(WhhT[:], WhhT_d.rearrange("(c p) n -> p c n", p=128))
        bih = const.tile([1, GW], f32, tag="bih")
        nc.sync.dma_start(bih[:], bih_d[:])
        bhh = const.tile([1, GW], f32, tag="bhh")
        nc.sync.dma_start(bhh[:], bhh_d[:])
        fcb = const.tile([1, VL], f32, tag="fcb")
        nc.sync.dma_start(fcb[:].bitcast(f32r), fcb_d[:].bitcast(f32r))
        ones1 = const.tile([1, 128], f32, tag="ones1")
        nc.any.memset(ones1[:], 1.0)
        onesr = const.tile([1, 128], f32, tag="onesr")
        nc.sync.dma_start(onesr[:].bitcast(f32r), ones_d[:].bitcast(f32r))

        # ---- prefetch all fc weight tiles (consumed in the fc loop) ----
        fcw_tiles = []
        for t in range(VL // NT):
            fcw = fcwp.tile([128, HC, NT], f32, tag="fcw")
            nc.sync.dma_start(
                fcw[:].bitcast(f32r),
                fcWT_d[t].rearrange("(c p) n -> p c n", p=128).bitcast(f32r))
            fcw_tiles.append(fcw)

        # ---- hW[b, h] = h0_loc @ W_h.T,   hwb = [hW ; attn_b] ----
        hwb = const.tile([BL + 1, H], f32, tag="hwb")
        nc.sync.dma_start(hwb[BL:BL + 1, :].bitcast(f32r), attnb_d[:].bitcast(f32r))
        p_hw = psum.tile([BL, H], f32, tag="mm")
        for c in range(HC):
            nc.tensor.matmul(
                p_hw[:], h0Tl[:, c, :].bitcast(f32r), WhT[:, c, :].bitcast(f32r),
                start=(c == 0), stop=(c == HC - 1))
        nc.vector.tensor_copy(hwb[0:BL, :].bitcast(f32r), p_hw[:])

        # ---- energyT[h, bs] = tanh(W_e @ encT + hW[b] + attn_b) ----
        et = const.tile([128, HC, 2, 512], f32, tag="et")  # [h-chunk][n-tile]
        for m in range(HC):
            for n in range(2):
                pe = psum.tile([128, 512], f32, tag="mm")
                for c in range(HC):
                    nc.tensor.matmul(
                        pe[:],
                        WeT[:, c, m * 128:(m + 1) * 128].bitcast(f32r),
                        encT[:, c, n * 512:(n + 1) * 512].bitcast(f32r),
                        start=(c == 0), stop=False)
                nc.tensor.matmul(
                    pe[:],
                    hwb[:, m * 128:(m + 1) * 128].bitcast(f32r),
                    selt[:, n * 512:(n + 1) * 512].bitcast(f32r),
                    start=False, stop=True)
                nc.scalar.activation(et[:, m, n, :].bitcast(f32r), pe[:], TANH)

        # ---- scores[bs] = sum_h energyT[h, bs] * v[h]  (psum [1, 512] x2) ----
        scores = work.tile([1, BS], f32, tag="scores")
        for n in range(2):
            ps = psum.tile([1, 512], f32, tag="mm")
            for m in range(HC):
                nc.tensor.matmul(
                    ps[:], v4[:, m:m + 1].bitcast(f32r),
                    et[:, m, n, :].bitcast(f32r),
                    start=(m == 0), stop=(m == HC - 1))
            nc.vector.tensor_copy(scores[:, n * 512:(n + 1) * 512].bitcast(f32r), ps[:])

        # ---- softmax over s (no max-sub; scores are tanh-bounded) ----
        # broadcast scores across partitions with a K=1 ones matmul, then exp
        expbc = work.tile([128, BS], f32, tag="expbc")
        for n in range(2):
            pw = psum.tile([128, 512], f32, tag="mm")
            nc.tensor.matmul(
                pw[:], onesr[:].bitcast(f32r),
                scores[:, n * 512:(n + 1) * 512].bitcast(f32r),
                start=True, stop=True)
            nc.scalar.activation(expbc[:, n * 512:(n + 1) * 512], pw[:], EXP)
        sums = work.tile([128, BL], f32, tag="sums")
        nc.vector.tensor_reduce(
            sums[:], expbc[:].rearrange("p (g s) -> p g s", s=S),
            axis=mybir.AxisListType.X, op=ADD)
        recip = work.tile([128, BL], f32, tag="recip")
        nc.vector.reciprocal(recip[:], sums[:])

        # ---- ctxT[d, b] = (sum_s encT * exp) / sum_exp ----
        ctxT = const.tile([128, HC, BL], f32, tag="ctxT")
        for c in range(HC):
            prod = work.tile([128, BS], f32, tag="prod")
            nc.vector.tensor_mul(prod[:], encT[:, c, :], expbc[:])
            raw = work.tile([128, BL], f32, tag="raw")
            nc.vector.tensor_reduce(
                raw[:], prod[:].rearrange("p (g s) -> p g s", s=S),
                axis=mybir.AxisListType.X, op=ADD)
            nc.vector.tensor_mul(ctxT[:, c, :], raw[:], recip[:])

        # ---- all-gather ctxT shards -> full [512, 128] ----
        ctx_in = dram.tile([H, BL], f32, tag="ctx_in")
        nc.sync.dma_start(
            ctx_in[:].rearrange("(c p) j -> p c j", p=128), ctxT[:])
        ctx_all = dram.tile([NC * H, BL], f32, tag="ctx_all")
        nc.gpsimd.collective_compute(
            "AllGather", mybir.AluOpType.bypass,
            replica_groups=[list(range(NC))],
            ins=[ctx_in[:].opt()], outs=[ctx_all[:].opt()])
        ctxg = const.tile([128, HC, NC, BL], f32, tag="ctxg")
        ctx_all_v = ctx_all[:].rearrange("(k c p) j -> c p k j", p=128, c=HC)
        for c in range(HC):
            nc.sync.dma_start(ctxg[:, c, :, :], ctx_all_v[c])

        # ---- gates^T shard [2*UL, B]: pg0 = [i;f] rows, pg1 = [g;o] rows ----
        bsum = work.tile([1, GW], f32, tag="bsum")
        nc.vector.tensor_add(bsum[:], bih[:], bhh[:])
        pg = []
        for half in range(2):
            lo, hi = half * 128, (half + 1) * 128
            p_g = psg.tile([128, B], f32, tag="pg")
            for c in range(HC):
                nc.tensor.matmul(p_g[:], WihT[:, c, lo:hi], embT[:, c, :],
                                 start=(c == 0), stop=False)
            for c in range(HC):
                nc.tensor.matmul(p_g[:], WihT[:, HC + c, lo:hi],
                                 ctxg[:, c, :, :], start=False, stop=False)
            for c in range(HC):
                nc.tensor.matmul(p_g[:], WhhT[:, c, lo:hi], h0Tf[:, c, :],
                                 start=False, stop=False)
            nc.tensor.matmul(p_g[:], bsum[:, lo:hi], ones1[:],
                             start=False, stop=True)
            pg.append(p_g)

        # ---- LSTM cell elementwise (unit-shard, transposed [u, b]) ----
        # each gate in its own base-0 tile: DVE needs equal base partitions
        si = work.tile([UL, B], f32, tag="si")
        nc.scalar.activation(si[:], pg[0][0:UL, :], SIG)
        sf = work.tile([UL, B], f32, tag="sf")
        nc.scalar.activation(sf[:], pg[0][UL:128, :], SIG)
        tg = work.tile([UL, B], f32, tag="tg")
        nc.scalar.activation(tg[:], pg[1][0:UL, :], TANH)
        so = work.tile([UL, B], f32, tag="so")
        nc.scalar.activation(so[:], pg[1][UL:128, :], SIG)
        t1 = work.tile([UL, B], f32, tag="t1")
        nc.vector.tensor_mul(t1[:], sf[:], c0T[:])
        t2 = work.tile([UL, B], f32, tag="t2")
        nc.vector.tensor_mul(t2[:], si[:], tg[:])
        cnew = work.tile([UL, B], f32, tag="cnew")
        nc.vector.tensor_add(cnew[:], t1[:], t2[:])
        nc.sync.dma_start(cT_d[:], cnew[:])
        tanc = work.tile([UL, B], f32, tag="tanc")
        nc.scalar.activation(tanc[:], cnew[:], TANH)
        hnew = work.tile([UL, B], f32, tag="hnew")
        nc.vector.tensor_mul(hnew[:], so[:], tanc[:])
        nc.sync.dma_start(hT_d[:], hnew[:])

        # ---- all-gather h_new^T shards -> full [512, 128] ----
        h_in = dram.tile([UL, B], f32, tag="h_in")
        nc.sync.dma_start(h_in[:], hnew[:])
        h_all = dram.tile([H, B], f32, tag="h_all")
        nc.gpsimd.collective_compute(
            "AllGather", mybir.AluOpType.bypass,
            replica_groups=[list(range(NC))],
            ins=[h_in[:].opt()], outs=[h_all[:].opt()])
        hT = const.tile([128, HC, B], f32, tag="hT")
        nc.sync.dma_start(hT[:].bitcast(f32r),
                          h_all[:].rearrange("(c p) b -> p c b", p=128).bitcast(f32r))

        # ---- fc: logits[b, v] = h_new @ fc_W.T + fc_b  (vocab shard) ----
        for t in range(VL // NT):
            fcw = fcw_tiles[t]
            pf = psum.tile([128, NT], f32, tag="mm")
            nc.tensor.matmul(pf[:], onesr[:].bitcast(f32r),
                             fcb[:, t * NT:(t + 1) * NT].bitcast(f32r),
                             start=True, stop=False)
            for c in range(HC):
                nc.tensor.matmul(pf[:], hT[:, c, :].bitcast(f32r),
                                 fcw[:, c, :].bitcast(f32r),
                                 start=False, stop=(c == HC - 1))
            lsb = outp.tile([128, NT], f32, tag="lsb")
            nc.vector.tensor_copy(lsb[:], pf[:])
            nc.sync.dma_start(logits_d[t], lsb[:])

    nc.compile()
    return nc


def _get_nc():
    if "nc" not in _CACHE:
        _CACHE["nc"] = _build()
    return _CACHE["nc"]


def _prep_in_maps(inputs):
    ids = np.asarray(inputs["input_ids"]).astype(np.int64)
    emb = np.asarray(inputs["emb"], dtype=np.float32)
    embT = np.ascontiguousarray(emb[ids].T)                    # [512, 128]
    h0 = np.asarray(inputs["h0"], dtype=np.float32)[0]         # [128, 512]
    h0T = np.ascontiguousarray(h0.T)                           # [512, 128]
    c0T = np.ascontiguousarray(np.asarray(inputs["c0"], dtype=np.float32)[0].T)
    enc = np.asarray(inputs["encoder_outputs"], dtype=np.float32)
    attn_W = np.asarray(inputs["attn_W"], dtype=np.float32)    # [512, 1024]
    WhT = np.ascontiguousarray(attn_W[:, :H].T)
    WeT = np.ascontiguousarray(attn_W[:, H:].T)
    attnb = np.asarray(inputs["attn_b"], dtype=np.float32).reshape(1, H)
    v4 = np.asarray(inputs["v"], dtype=np.float32).reshape(HC, 128)
    W_ih = np.asarray(inputs["W_ih"], dtype=np.float32)        # [2048, 1024]
    W_hh = np.asarray(inputs["W_hh"], dtype=np.float32)        # [2048, 512]
    b_ih = np.asarray(inputs["b_ih"], dtype=np.float32)
    b_hh = np.asarray(inputs["b_hh"], dtype=np.float32)
    fc_W = np.asarray(inputs["fc_W"], dtype=np.float32)        # [32000, 512]
    fc_b = np.asarray(inputs["fc_b"], dtype=np.float32)

    in_maps = []
    for k in range(NC):
        rows = np.concatenate([np.arange(g * H + k * UL, g * H + (k + 1) * UL)
                               for g in range(4)])             # i,f,g,o shard
        encT_k = np.ascontiguousarray(
            enc[k * BL:(k + 1) * BL].reshape(BS, H).T)         # [512, 1024]
        fcWT_k = np.ascontiguousarray(
            fc_W[k * VL:(k + 1) * VL].T.reshape(H, VL // NT, NT)
            .transpose(1, 0, 2))                               # [8, 512, 500]
        in_maps.append({
            "encT": encT_k,
            "WeT": WeT,
            "WhT": WhT,
            "h0Tl": np.ascontiguousarray(h0T[:, k * BL:(k + 1) * BL]),
            "h0Tf": h0T,
            "embT": embT,
            "attnb": attnb,
            "v4": v4,
            "c0T": np.ascontiguousarray(c0T[k * UL:(k + 1) * UL]),
            "WihT": np.ascontiguousarray(W_ih[rows].T),        # [1024, 256]
            "WhhT": np.ascontiguousarray(W_hh[rows].T),        # [512, 256]
            "bih": b_ih[rows].reshape(1, GW).copy(),
            "bhh": b_hh[rows].reshape(1, GW).copy(),
            "fcWT": fcWT_k,
            "fcb": fc_b[k * VL:(k + 1) * VL].reshape(1, VL).copy(),
        })
    return in_maps


def _assemble(results):
    pred = np.empty((B, V), np.float32)
    for k in range(NC):
        pred[:, k * VL:(k + 1) * VL] = (
            results[k]["logits"].transpose(1, 0, 2).reshape(B, VL))
    hT = np.concatenate([results[k]["hT"] for k in range(NC)], axis=0)
    cT = np.concatenate([results[k]["cT"] for k in range(NC)], axis=0)
    return pred, np.ascontiguousarray(hT.T)[None], np.ascontiguousarray(cT.T)[None]


def kernel(**inputs):
    from concourse.bass_utils import run_bass_kernel_spmd

    nc = _get_nc()
    in_maps = _prep_in_maps(inputs)
    res = run_bass_kernel_spmd(nc, in_maps, core_ids=list(range(NC)))
    return _assemble(res.results)


# revision 11
# speedup vs baseline: 1.1492x; 1.1492x over previous
"""DecoderRNN (attention + LSTM cell + vocab projection) on 8 TRN2 NeuronCores.

Sharding:
  - Attention: data-parallel over batch (16 rows/core), computed in a
    transposed [feature, batch*seq] layout so every matmul contracts over
    the partition dim.
  - LSTM cell: tensor-parallel over the 4H gate dim -- core k owns hidden
    units [64k, 64k+64) of each of i/f/g/o, for the full batch.  Needs the
    full rnn_in, so the per-core context shards are all-gathered first.
  - FC (vocab): tensor-parallel over V (4000 rows/core) after all-gathering
    the h_new shards.
Outputs per core: logits slab [8,128,500], hT/cT shards [64,128]; the host
reassembles the full (prediction, h_new, c_new).
"""

import sys

if "/opt/trn_rl_repo" not in sys.path:
    sys.path.insert(0, "/opt/trn_rl_repo")

import numpy as np

V, H, B, S = 32000, 512, 128, 64
NC = 8            # cores
BL = B // NC      # 16  local batch rows (attention shard)
UL = H // NC      # 64  local hidden units (lstm shard)
VL = V // NC      # 4000 local vocab rows (fc shard)
GW = 4 * UL       # 256 local gate rows (4 gates x 64 units)
NT = 500          # fc free-dim tile (8 tiles of 500 = 4000)
HC = H // 128     # 4   128-row chunks of H
DC = 2 * H // 128  # 8  128-row chunks of 2H (rnn_in)
BS = BL * S       # 1024 local (batch, seq) positions

_CACHE = {}


def _build():
    import concourse.bass as bass
    import concourse.bacc as bacc
    import concourse.mybir as mybir
    import concourse.tile as tile
    from contextlib import ExitStack

    f32 = mybir.dt.float32
    f32r = mybir.dt.float32r
    TANH = mybir.ActivationFunctionType.Tanh
    SIG = mybir.ActivationFunctionType.Sigmoid
    EXP = mybir.ActivationFunctionType.Exp
    ADD = mybir.AluOpType.add

    nc = bacc.Bacc("TRN2", target_bir_lowering=False, debug=False, num_devices=NC)

    def din(name, shape):
        return nc.dram_tensor(name, shape, f32, kind="ExternalInput").ap()

    def dout(name, shape):
        return nc.dram_tensor(name, shape, f32, kind="ExternalOutput").ap()

    encT_d = din("encT", [H, BS])          # enc shard, transposed [d, b*s]
    WeT_d = din("WeT", [H, H])             # attn_W[:,H:].T   [d, h]
    WhT_d = din("WhT", [H, H])             # attn_W[:,:H].T   [d, h]
    h0Tl_d = din("h0Tl", [H, BL])          # local h0.T slice [d, bl]
    h0Tf_d = din("h0Tf", [H, B])           # full h0.T (replicated)
    embT_d = din("embT", [H, B])           # full embedded.T (replicated)
    attnb_d = din("attnb", [1, H])
    v4_d = din("v4", [HC, 128])            # v reshaped [4,128]
    c0T_d = din("c0T", [UL, B])            # local c0.T slice (unit shard)
    WihT_d = din("WihT", [2 * H, GW])  # W_ih rows(shard).T  [1024, 256]
    WhhT_d = din("WhhT", [H, GW])      # W_hh rows(shard).T  [512, 256]
    bih_d = din("bih", [1, GW])
    bhh_d = din("bhh", [1, GW])
    fcWT_d = din("fcWT", [V // VL, H, NT])  # [8, 512, 500] per-tile contiguous
    fcb_d = din("fcb", [1, VL])

    logits_d = dout("logits", [VL // NT, B, NT])  # [8, 128, 500]
    hT_d = dout("hT", [UL, B])
    cT_d = dout("cT", [UL, B])

    # SEL[b, bs] = 1 if bs // S == b ; SEL[BL, :] = 1  (adds attn_b row)
    sel_np = np.zeros((BL + 1, BS), np.float32)
    for b in range(BL):
        sel_np[b, b * S:(b + 1) * S] = 1.0
    sel_np[BL, :] = 1.0
    sel_d = nc.inline_tensor(sel_np, name="sel").ap()
    ones_d = nc.inline_tensor(np.ones((1, 128), np.float32), name="onesc").ap()

    with tile.TileContext(nc) as tc, ExitStack() as ctx:
        const = ctx.enter_context(tc.tile_pool(name="const", bufs=1))
        work = ctx.enter_context(tc.tile_pool(name="work", bufs=2))
        fcwp = ctx.enter_context(tc.tile_pool(name="fcw", bufs=8))
        outp = ctx.enter_context(tc.tile_pool(name="outp", bufs=3))
        psum = ctx.enter_context(tc.tile_pool(name="psum", bufs=6, space="PSUM"))
        psg = ctx.enter_context(tc.tile_pool(name="psg", bufs=2, space="PSUM"))
        dram = ctx.enter_context(tc.tile_pool(name="dram", bufs=1, space="DRAM"))

        # ---- constant / weight loads (attention-critical first) ----
        h0Tl = const.tile([128, HC, BL], f32, tag="h0Tl")
        nc.sync.dma_start(h0Tl[:].bitcast(f32r),
                          h0Tl_d.rearrange("(c p) n -> p c n", p=128).bitcast(f32r))
        WhT = const.tile([128, HC, H], f32, tag="WhT")
        nc.sync.dma_start(WhT[:].bitcast(f32r),
                          WhT_d.rearrange("(c p) n -> p c n", p=128).bitcast(f32r))
        WeT = const.tile([128, HC, H], f32, tag="WeT")
        nc.sync.dma_start(WeT[:].bitcast(f32r),
                          WeT_d.rearrange("(c p) n -> p c n", p=128).bitcast(f32r))
        encT = const.tile([128, HC, BS], f32, tag="encT")
        nc.sync.dma_start(encT[:].bitcast(f32r),
                          encT_d.rearrange("(c p) n -> p c n", p=128).bitcast(f32r))
        selt = const.tile([BL + 1, BS], f32, tag="selt")
        nc.sync.dma_start(selt[:].bitcast(f32r), sel_d[:].bitcast(f32r))
        hwb = const.tile([BL + 1, H], f32, tag="hwb")
        nc.sync.dma_start(hwb[BL:BL + 1, :].bitcast(f32r), attnb_d[:].bitcast(f32r))
        v4 = const.tile([128, HC], f32, tag="v4")
        nc.sync.dma_start(v4[:].bitcast(f32r), v4_d.rearrange("c p -> p c").bitcast(f32r))
        onesr = const.tile([1, 128], f32, tag="onesr")
        nc.sync.dma_start(onesr[:].bitcast(f32r), ones_d[:].bitcast(f32r))
        ones1 = const.tile([1, 128], f32, tag="ones1")
        nc.any.memset(ones1[:], 1.0)
        # lstm inputs (needed mid-kernel)
        WihT = const.tile([128, DC, GW], f32, tag="WihT")
        nc.sync.dma_start(WihT[:], WihT_d.rearrange("(c p) n -> p c n", p=128))
        WhhT = const.tile([128, HC, GW], f32, tag="WhhT")
        nc.sync.dma_start(WhhT[:], WhhT_d.rearrange("(c p) n -> p c n", p=128))
        h0Tf = const.tile([128, HC, B], f32, tag="h0Tf")
        nc.sync.dma_start(h0Tf[:], h0Tf_d.rearrange("(c p) n -> p c n", p=128))
        embT = const.tile([128, HC, B], f32, tag="embT")
        nc.sync.dma_start(embT[:], embT_d.rearrange("(c p) n -> p c n", p=128))
        bih = const.tile([1, GW], f32, tag="bih")
        nc.sync.dma_start(bih[:], bih_d[:])
        bhh = const.tile([1, GW], f32, tag="bhh")
        nc.sync.dma_start(bhh[:], bhh_d[:])
        c0T = const.tile([UL, B], f32, tag="c0T")
        nc.sync.dma_start(c0T[:], c0T_d[:])
        fcb = const.tile([1, VL], f32, tag="fcb")
        nc.sync.dma_start(fcb[:].bitcast(f32r), fcb_d[:].bitcast(f32r))

        # ---- prefetch all fc weight tiles (consumed in the fc loop) ----
        fcw_tiles = []
        for t in range(VL // NT):
            fcw = fcwp.tile([128, HC, NT], f32, tag="fcw")
            nc.sync.dma_start(
                fcw[:].bitcast(f32r),
                fcWT_d[t].rearrange("(c p) n -> p c n", p=128).bitcast(f32r))
            fcw_tiles.append(fcw)

        # ---- hW[b, h] = h0_loc @ W_h.T,   hwb = [hW ; attn_b] ----
        p_hw = psum.tile([BL, H], f32, tag="mm")
        for c in range(HC):
            nc.tensor.matmul(
                p_hw[:], h0Tl[:, c, :].bitcast(f32r), WhT[:, c, :].bitcast(f32r),
                start=(c == 0), stop=(c == HC - 1))
        nc.vector.tensor_copy(hwb[0:BL, :].bitcast(f32r), p_hw[:])

        # ---- energyT[h, bs] = tanh(W_e @ encT + hW[b] + attn_b) ----
        et = const.tile([128, HC, 2, 512], f32, tag="et")  # [h-chunk][n-tile]
        for m in range(HC):
            for n in range(2):
                pe = psum.tile([128, 512], f32, tag="mm")
                for c in range(HC):
                    nc.tensor.matmul(
                        pe[:],
                        WeT[:, c, m * 128:(m + 1) * 128].bitcast(f32r),
                        encT[:, c, n * 512:(n + 1) * 512].bitcast(f32r),
                        start=(c == 0), stop=False)
                nc.tensor.matmul(
                    pe[:],
                    hwb[:, m * 128:(m + 1) * 128].bitcast(f32r),
                    selt[:, n * 512:(n + 1) * 512].bitcast(f32r),
                    start=False, stop=True)
                nc.scalar.activation(et[:, m, n, :].bitcast(f32r), pe[:], TANH)

        # ---- scores[bs] = sum_h energyT[h, bs] * v[h]  (psum [1, 512] x2) ----
        scores = work.tile([1, BS], f32, tag="scores")
        for n in range(2):
            ps = psum.tile([1, 512], f32, tag="mm")
            for m in range(HC):
                nc.tensor.matmul(
                    ps[:], v4[:, m:m + 1].bitcast(f32r),
                    et[:, m, n, :].bitcast(f32r),
                    start=(m == 0), stop=(m == HC - 1))
            nc.vector.tensor_copy(scores[:, n * 512:(n + 1) * 512].bitcast(f32r), ps[:])

        # ---- softmax over s (no max-sub; scores are tanh-bounded) ----
        # broadcast scores across partitions with a K=1 ones matmul, then exp
        expbc = work.tile([128, BS], f32, tag="expbc")
        for n in range(2):
            pw = psum.tile([128, 512], f32, tag="mm")
            nc.tensor.matmul(
                pw[:], onesr[:].bitcast(f32r),
                scores[:, n * 512:(n + 1) * 512].bitcast(f32r),
                start=True, stop=True)
            nc.scalar.activation(expbc[:, n * 512:(n + 1) * 512], pw[:], EXP)
        sums = work.tile([128, BL], f32, tag="sums")
        nc.vector.tensor_reduce(
            sums[:], expbc[:].rearrange("p (g s) -> p g s", s=S),
            axis=mybir.AxisListType.X, op=ADD)
        recip = work.tile([128, BL], f32, tag="recip")
        nc.vector.reciprocal(recip[:], sums[:])

        # ---- ctxT[d, b] = (sum_s encT * exp) / sum_exp ----
        ctxT = const.tile([128, HC, BL], f32, tag="ctxT")
        for c in range(HC):
            prod = work.tile([128, BS], f32, tag="prod")
            nc.vector.tensor_mul(prod[:], encT[:, c, :], expbc[:])
            raw = work.tile([128, BL], f32, tag="raw")
            nc.vector.tensor_reduce(
                raw[:], prod[:].rearrange("p (g s) -> p g s", s=S),
                axis=mybir.AxisListType.X, op=ADD)
            nc.vector.tensor_mul(ctxT[:, c, :], raw[:], recip[:])

        # ---- gates pre-accumulation (emb, h0, bias): overlaps ctx gather ----
        bsum = work.tile([1, GW], f32, tag="bsum")
        nc.vector.tensor_add(bsum[:], bih[:], bhh[:])
        pg = []
        for half in range(2):
            lo, hi = half * 128, (half + 1) * 128
            p_g = psg.tile([128, B], f32, tag="pg")
            for c in range(HC):
                nc.tensor.matmul(p_g[:], WihT[:, c, lo:hi], embT[:, c, :],
                                 start=(c == 0), stop=False)
            for c in range(HC):
                nc.tensor.matmul(p_g[:], WhhT[:, c, lo:hi], h0Tf[:, c, :],
                                 start=False, stop=False)
            nc.tensor.matmul(p_g[:], bsum[:, lo:hi], ones1[:],
                             start=False, stop=False)
            pg.append(p_g)

        # ---- all-gather ctxT shards -> full [512, 128] ----
        ctx_in = dram.tile([H, BL], f32, tag="ctx_in")
        nc.sync.dma_start(
            ctx_in[:].rearrange("(c p) j -> p c j", p=128), ctxT[:])
        ctx_all = dram.tile([NC * H, BL], f32, tag="ctx_all")
        nc.gpsimd.collective_compute(
            "AllGather", mybir.AluOpType.bypass,
            replica_groups=[list(range(NC))],
            ins=[ctx_in[:].opt()], outs=[ctx_all[:].opt()])
        ctxg = const.tile([128, HC, NC, BL], f32, tag="ctxg")
        ctx_all_v = ctx_all[:].rearrange("(k c p) j -> c p k j", p=128, c=HC)
        for c in range(HC):
            nc.sync.dma_start(ctxg[:, c, :, :], ctx_all_v[c])

        # ---- gates: ctx-dependent accumulation (after the gather) ----
        for half in range(2):
            lo, hi = half * 128, (half + 1) * 128
            for c in range(HC):
                nc.tensor.matmul(pg[half][:], WihT[:, HC + c, lo:hi],
                                 ctxg[:, c, :, :], start=False,
                                 stop=(c == HC - 1))

        # ---- LSTM cell elementwise (unit-shard, transposed [u, b]) ----
        # each gate in its own base-0 tile: DVE needs equal base partitions
        si = work.tile([UL, B], f32, tag="si")
        nc.scalar.activation(si[:], pg[0][0:UL, :], SIG)
        sf = work.tile([UL, B], f32, tag="sf")
        nc.scalar.activation(sf[:], pg[0][UL:128, :], SIG)
        tg = work.tile([UL, B], f32, tag="tg")
        nc.scalar.activation(tg[:], pg[1][0:UL, :], TANH)
        so = work.tile([UL, B], f32, tag="so")
        nc.scalar.activation(so[:], pg[1][UL:128, :], SIG)
        t1 = work.tile([UL, B], f32, tag="t1")
        nc.vector.tensor_mul(t1[:], sf[:], c0T[:])
        t2 = work.tile([UL, B], f32, tag="t2")
        nc.vector.tensor_mul(t2[:], si[:], tg[:])
        cnew = work.tile([UL, B], f32, tag="cnew")
        nc.vector.tensor_add(cnew[:], t1[:], t2[:])
        nc.sync.dma_start(cT_d[:], cnew[:])
        tanc = work.tile([UL, B], f32, tag="tanc")
        nc.scalar.activation(tanc[:], cnew[:], TANH)
        hnew = work.tile([UL, B], f32, tag="hnew")
        nc.vector.tensor_mul(hnew[:], so[:], tanc[:])
        nc.sync.dma_start(hT_d[:], hnew[:])

        # ---- all-gather h_new^T shards -> full [512, 128] ----
        h_in = dram.tile([UL, B], f32, tag="h_in")
        nc.sync.dma_start(h_in[:], hnew[:])
        h_all = dram.tile([H, B], f32, tag="h_all")
        nc.gpsimd.collective_compute(
            "AllGather", mybir.AluOpType.bypass,
            replica_groups=[list(range(NC))],
            ins=[h_in[:].opt()], outs=[h_all[:].opt()])
        hT = const.tile([128, HC, B], f32, tag="hT")
        nc.sync.dma_start(hT[:].bitcast(f32r),
                          h_all[:].rearrange("(c p) b -> p c b", p=128).bitcast(f32r))

        # ---- fc: logits[b, v] = h_new @ fc_W.T + fc_b  (vocab shard) ----
        # bias prefill for the first 6 tiles overlaps the h gather
        NPRE = 6
        pf_tiles = {}
        for t in range(NPRE):
            pf = psum.tile([128, NT], f32, tag="mm")
            nc.tensor.matmul(pf[:], onesr[:].bitcast(f32r),
                             fcb[:, t * NT:(t + 1) * NT].bitcast(f32r),
                             start=True, stop=False)
            pf_tiles[t] = pf
        for t in range(VL // NT):
            fcw = fcw_tiles[t]
            if t < NPRE:
                pf = pf_tiles[t]
            else:
                pf = psum.tile([128, NT], f32, tag="mm")
                nc.tensor.matmul(pf[:], onesr[:].bitcast(f32r),
                                 fcb[:, t * NT:(t + 1) * NT].bitcast(f32r),
                                 start=True, stop=False)
            for c in range(HC):
                nc.tensor.matmul(pf[:], hT[:, c, :].bitcast(f32r),
                                 fcw[:, c, :].bitcast(f32r),
                                 start=False, stop=(c == HC - 1))
            lsb = outp.tile([128, NT], f32, tag="lsb")
            nc.vector.tensor_copy(lsb[:], pf[:])
            nc.sync.dma_start(logits_d[t], lsb[:])

    nc.compile()
    return nc


def _get_nc():
    if "nc" not in _CACHE:
        _CACHE["nc"] = _build()
    return _CACHE["nc"]


def _prep_in_maps(inputs):
    ids = np.asarray(inputs["input_ids"]).astype(np.int64)
    emb = np.asarray(inputs["emb"], dtype=np.float32)
    embT = np.ascontiguousarray(emb[ids].T)                    # [512, 128]
    h0 = np.asarray(inputs["h0"], dtype=np.float32)[0]         # [128, 512]
    h0T = np.ascontiguousarray(h0.T)                           # [512, 128]
    c0T = np.ascontiguousarray(np.asarray(inputs["c0"], dtype=np.float32)[0].T)
    enc = np.asarray(inputs["encoder_outputs"], dtype=np.float32)
    attn_W = np.asarray(inputs["attn_W"], dtype=np.float32)    # [512, 1024]
    WhT = np.ascontiguousarray(attn_W[:, :H].T)
    WeT = np.ascontiguousarray(attn_W[:, H:].T)
    attnb = np.asarray(inputs["attn_b"], dtype=np.float32).reshape(1, H)
    v4 = np.asarray(inputs["v"], dtype=np.float32).reshape(HC, 128)
    W_ih = np.asarray(inputs["W_ih"], dtype=np.float32)        # [2048, 1024]
    W_hh = np.asarray(inputs["W_hh"], dtype=np.float32)        # [2048, 512]
    b_ih = np.asarray(inputs["b_ih"], dtype=np.float32)
    b_hh = np.asarray(inputs["b_hh"], dtype=np.float32)
    fc_W = np.asarray(inputs["fc_W"], dtype=np.float32)        # [32000, 512]
    fc_b = np.asarray(inputs["fc_b"], dtype=np.float32)

    in_maps = []
    for k in range(NC):
        rows = np.concatenate([np.arange(g * H + k * UL, g * H + (k + 1) * UL)
                               for g in range(4)])             # i,f,g,o shard
        encT_k = np.ascontiguousarray(
            enc[k * BL:(k + 1) * BL].reshape(BS, H).T)         # [512, 1024]
        fcWT_k = np.ascontiguousarray(
            fc_W[k * VL:(k + 1) * VL].T.reshape(H, VL // NT, NT)
            .transpose(1, 0, 2))                               # [8, 512, 500]
        in_maps.append({
            "encT": encT_k,
            "WeT": WeT,
            "WhT": WhT,
            "h0Tl": np.ascontiguousarray(h0T[:, k * BL:(k + 1) * BL]),
            "h0Tf": h0T,
            "embT": embT,
            "attnb": attnb,
            "v4": v4,
            "c0T": np.ascontiguousarray(c0T[k * UL:(k + 1) * UL]),
            "WihT": np.ascontiguousarray(W_ih[rows].T),        # [1024, 256]
            "WhhT": np.ascontiguousarray(W_hh[rows].T),        # [512, 256]
            "bih": b_ih[rows].reshape(1, GW).copy(),
            "bhh": b_hh[rows].reshape(1, GW).copy(),
            "fcWT": fcWT_k,
            "fcb": fc_b[k * VL:(k + 1) * VL].reshape(1, VL).copy(),
        })
    return in_maps


def _assemble(results):
    pred = np.empty((B, V), np.float32)
    for k in range(NC):
        pred[:, k * VL:(k + 1) * VL] = (
            results[k]["logits"].transpose(1, 0, 2).reshape(B, VL))
    hT = np.concatenate([results[k]["hT"] for k in range(NC)], axis=0)
    cT = np.concatenate([results[k]["cT"] for k in range(NC)], axis=0)
    return pred, np.ascontiguousarray(hT.T)[None], np.ascontiguousarray(cT.T)[None]


def kernel(**inputs):
    from concourse.bass_utils import run_bass_kernel_spmd

    nc = _get_nc()
    in_maps = _prep_in_maps(inputs)
    res = run_bass_kernel_spmd(nc, in_maps, core_ids=list(range(NC)))
    return _assemble(res.results)


# revision 12
# speedup vs baseline: 1.1907x; 1.0362x over previous
"""DecoderRNN (attention + LSTM cell + vocab projection) on 8 TRN2 NeuronCores.

Sharding:
  - Attention: data-parallel over batch (16 rows/core), computed in a
    transposed [feature, batch*seq] layout so every matmul contracts over
    the partition dim.
  - LSTM cell: tensor-parallel over the 4H gate dim -- core k owns hidden
    units [64k, 64k+64) of each of i/f/g/o, for the full batch.  Needs the
    full rnn_in, so the per-core context shards are all-gathered first.
  - FC (vocab): tensor-parallel over V (4000 rows/core) after all-gathering
    the h_new shards.
Outputs per core: logits slab [8,128,500], hT/cT shards [64,128]; the host
reassembles the full (prediction, h_new, c_new).
"""

import sys

if "/opt/trn_rl_repo" not in sys.path:
    sys.path.insert(0, "/opt/trn_rl_repo")

import numpy as np

V, H, B, S = 32000, 512, 128, 64
NC = 8            # cores
BL = B // NC      # 16  local batch rows (attention shard)
UL = H // NC      # 64  local hidden units (lstm shard)
VL = V // NC      # 4000 local vocab rows (fc shard)
GW = 4 * UL       # 256 local gate rows (4 gates x 64 units)
NT = 500          # fc free-dim tile (8 tiles of 500 = 4000)
HC = H // 128     # 4   128-row chunks of H
DC = 2 * H // 128  # 8  128-row chunks of 2H (rnn_in)
BS = BL * S       # 1024 local (batch, seq) positions

_CACHE = {}


def _build():
    import concourse.bass as bass
    import concourse.bacc as bacc
    import concourse.mybir as mybir
    import concourse.tile as tile
    from contextlib import ExitStack

    f32 = mybir.dt.float32
    f32r = mybir.dt.float32r
    TANH = mybir.ActivationFunctionType.Tanh
    SIG = mybir.ActivationFunctionType.Sigmoid
    EXP = mybir.ActivationFunctionType.Exp
    ADD = mybir.AluOpType.add

    nc = bacc.Bacc("TRN2", target_bir_lowering=False, debug=False, num_devices=NC)

    def din(name, shape):
        return nc.dram_tensor(name, shape, f32, kind="ExternalInput").ap()

    def dout(name, shape):
        return nc.dram_tensor(name, shape, f32, kind="ExternalOutput").ap()

    encT_d = din("encT", [H, BS])          # enc shard, transposed [d, b*s]
    WeT_d = din("WeT", [H, H])             # attn_W[:,H:].T   [d, h]
    WhT_d = din("WhT", [H, H])             # attn_W[:,:H].T   [d, h]
    h0Tl_d = din("h0Tl", [H, BL])          # local h0.T slice [d, bl]
    h0Tf_d = din("h0Tf", [H, B])           # full h0.T (replicated)
    embT_d = din("embT", [H, B])           # full embedded.T (replicated)
    attnb_d = din("attnb", [1, H])
    v4_d = din("v4", [HC, 128])            # v reshaped [4,128]
    c0T_d = din("c0T", [UL, B])            # local c0.T slice (unit shard)
    WihT_d = din("WihT", [2 * H, GW])  # W_ih rows(shard).T  [1024, 256]
    WhhT_d = din("WhhT", [H, GW])      # W_hh rows(shard).T  [512, 256]
    bih_d = din("bih", [1, GW])
    bhh_d = din("bhh", [1, GW])
    fcWT_d = din("fcWT", [V // VL, H, NT])  # [8, 512, 500] per-tile contiguous
    fcb_d = din("fcb", [1, VL])

    logits_d = dout("logits", [VL // NT, B, NT])  # [8, 128, 500]
    hT_d = dout("hT", [UL, B])
    cT_d = dout("cT", [UL, B])

    # SEL[b, bs] = 1 if bs // S == b ; SEL[BL, :] = 1  (adds attn_b row)
    sel_np = np.zeros((BL + 1, BS), np.float32)
    for b in range(BL):
        sel_np[b, b * S:(b + 1) * S] = 1.0
    sel_np[BL, :] = 1.0
    sel_d = nc.inline_tensor(sel_np, name="sel").ap()
    ones_d = nc.inline_tensor(np.ones((1, 128), np.float32), name="onesc").ap()

    with tile.TileContext(nc) as tc, ExitStack() as ctx:
        const = ctx.enter_context(tc.tile_pool(name="const", bufs=1))
        work = ctx.enter_context(tc.tile_pool(name="work", bufs=2))
        fcwp = ctx.enter_context(tc.tile_pool(name="fcw", bufs=8))
        outp = ctx.enter_context(tc.tile_pool(name="outp", bufs=3))
        psum = ctx.enter_context(tc.tile_pool(name="psum", bufs=6, space="PSUM"))
        psg = ctx.enter_context(tc.tile_pool(name="psg", bufs=2, space="PSUM"))
        dram = ctx.enter_context(tc.tile_pool(name="dram", bufs=1, space="DRAM"))

        # ---- constant / weight loads (attention-critical first) ----
        h0Tl = const.tile([128, HC, BL], f32, tag="h0Tl")
        nc.sync.dma_start(h0Tl[:].bitcast(f32r),
                          h0Tl_d.rearrange("(c p) n -> p c n", p=128).bitcast(f32r))
        WhT = const.tile([128, HC, H], f32, tag="WhT")
        WhT_v = WhT_d.rearrange("(c p) n -> c p n", p=128)
        for c in range(HC):
            nc.sync.dma_start(WhT[:, c, :].bitcast(f32r), WhT_v[c].bitcast(f32r))
        WeT = const.tile([128, HC, H], f32, tag="WeT")
        WeT_v = WeT_d.rearrange("(c p) n -> c p n", p=128)
        for c in range(HC):
            nc.sync.dma_start(WeT[:, c, :].bitcast(f32r), WeT_v[c].bitcast(f32r))
        encT = const.tile([128, HC, BS], f32, tag="encT")
        encT_v = encT_d.rearrange("(c p) n -> c p n", p=128)
        for c in range(HC):
            nc.sync.dma_start(encT[:, c, :].bitcast(f32r), encT_v[c].bitcast(f32r))
        selt = const.tile([BL + 1, BS], f32, tag="selt")
        nc.sync.dma_start(selt[:].bitcast(f32r), sel_d[:].bitcast(f32r))
        hwb = const.tile([BL + 1, H], f32, tag="hwb")
        nc.sync.dma_start(hwb[BL:BL + 1, :].bitcast(f32r), attnb_d[:].bitcast(f32r))
        v4 = const.tile([128, HC], f32, tag="v4")
        nc.sync.dma_start(v4[:].bitcast(f32r), v4_d.rearrange("c p -> p c").bitcast(f32r))
        onesr = const.tile([1, 128], f32, tag="onesr")
        nc.sync.dma_start(onesr[:].bitcast(f32r), ones_d[:].bitcast(f32r))
        ones1 = const.tile([1, 128], f32, tag="ones1")
        nc.any.memset(ones1[:], 1.0)
        # lstm inputs (needed mid-kernel)
        WihT = const.tile([128, DC, GW], f32, tag="WihT")
        nc.sync.dma_start(WihT[:], WihT_d.rearrange("(c p) n -> p c n", p=128))
        WhhT = const.tile([128, HC, GW], f32, tag="WhhT")
        nc.sync.dma_start(WhhT[:], WhhT_d.rearrange("(c p) n -> p c n", p=128))
        h0Tf = const.tile([128, HC, B], f32, tag="h0Tf")
        nc.sync.dma_start(h0Tf[:], h0Tf_d.rearrange("(c p) n -> p c n", p=128))
        embT = const.tile([128, HC, B], f32, tag="embT")
        nc.sync.dma_start(embT[:], embT_d.rearrange("(c p) n -> p c n", p=128))
        bih = const.tile([1, GW], f32, tag="bih")
        nc.sync.dma_start(bih[:], bih_d[:])
        bhh = const.tile([1, GW], f32, tag="bhh")
        nc.sync.dma_start(bhh[:], bhh_d[:])
        c0T = const.tile([UL, B], f32, tag="c0T")
        nc.sync.dma_start(c0T[:], c0T_d[:])
        fcb = const.tile([1, VL], f32, tag="fcb")
        nc.sync.dma_start(fcb[:].bitcast(f32r), fcb_d[:].bitcast(f32r))

        # ---- prefetch all fc weight tiles (consumed in the fc loop) ----
        fcw_tiles = []
        for t in range(VL // NT):
            fcw = fcwp.tile([128, HC, NT], f32, tag="fcw")
            nc.sync.dma_start(
                fcw[:].bitcast(f32r),
                fcWT_d[t].rearrange("(c p) n -> p c n", p=128).bitcast(f32r))
            fcw_tiles.append(fcw)

        # ---- hW[b, h] = h0_loc @ W_h.T,   hwb = [hW ; attn_b] ----
        p_hw = psum.tile([BL, H], f32, tag="mm")
        for c in range(HC):
            nc.tensor.matmul(
                p_hw[:], h0Tl[:, c, :].bitcast(f32r), WhT[:, c, :].bitcast(f32r),
                start=(c == 0), stop=(c == HC - 1))
        nc.vector.tensor_copy(hwb[0:BL, :].bitcast(f32r), p_hw[:])

        # ---- energyT[h, bs] = tanh(W_e @ encT + hW[b] + attn_b) ----
        et = const.tile([128, HC, 2, 512], f32, tag="et")  # [h-chunk][n-tile]
        for m in range(HC):
            for n in range(2):
                pe = psum.tile([128, 512], f32, tag="mm")
                for c in range(HC):
                    nc.tensor.matmul(
                        pe[:],
                        WeT[:, c, m * 128:(m + 1) * 128].bitcast(f32r),
                        encT[:, c, n * 512:(n + 1) * 512].bitcast(f32r),
                        start=(c == 0), stop=False)
                nc.tensor.matmul(
                    pe[:],
                    hwb[:, m * 128:(m + 1) * 128].bitcast(f32r),
                    selt[:, n * 512:(n + 1) * 512].bitcast(f32r),
                    start=False, stop=True)
                nc.scalar.activation(et[:, m, n, :].bitcast(f32r), pe[:], TANH)

        # ---- scores[bs] = sum_h energyT[h, bs] * v[h]  (psum [1, 512] x2) ----
        scores = work.tile([1, BS], f32, tag="scores")
        for n in range(2):
            ps = psum.tile([1, 512], f32, tag="mm")
            for m in range(HC):
                nc.tensor.matmul(
                    ps[:], v4[:, m:m + 1].bitcast(f32r),
                    et[:, m, n, :].bitcast(f32r),
                    start=(m == 0), stop=(m == HC - 1))
            nc.vector.tensor_copy(scores[:, n * 512:(n + 1) * 512].bitcast(f32r), ps[:])

        # ---- softmax over s (no max-sub; scores are tanh-bounded) ----
        # broadcast scores across partitions with a K=1 ones matmul, then exp
        expbc = work.tile([128, BS], f32, tag="expbc")
        for n in range(2):
            pw = psum.tile([128, 512], f32, tag="mm")
            nc.tensor.matmul(
                pw[:], onesr[:].bitcast(f32r),
                scores[:, n * 512:(n + 1) * 512].bitcast(f32r),
                start=True, stop=True)
            nc.scalar.activation(expbc[:, n * 512:(n + 1) * 512], pw[:], EXP)
        sums = work.tile([128, BL], f32, tag="sums")
        nc.vector.tensor_reduce(
            sums[:], expbc[:].rearrange("p (g s) -> p g s", s=S),
            axis=mybir.AxisListType.X, op=ADD)
        recip = work.tile([128, BL], f32, tag="recip")
        nc.vector.reciprocal(recip[:], sums[:])

        # ---- ctxT[d, b] = (sum_s encT * exp) / sum_exp ----
        ctxT = const.tile([128, HC, BL], f32, tag="ctxT")
        for c in range(HC):
            prod = work.tile([128, BS], f32, tag="prod")
            nc.vector.tensor_mul(prod[:], encT[:, c, :], expbc[:])
            raw = work.tile([128, BL], f32, tag="raw")
            nc.vector.tensor_reduce(
                raw[:], prod[:].rearrange("p (g s) -> p g s", s=S),
                axis=mybir.AxisListType.X, op=ADD)
            nc.vector.tensor_mul(ctxT[:, c, :], raw[:], recip[:])

        # ---- gates pre-accumulation (emb, h0, bias): overlaps ctx gather ----
        bsum = work.tile([1, GW], f32, tag="bsum")
        nc.vector.tensor_add(bsum[:], bih[:], bhh[:])
        pg = []
        for half in range(2):
            lo, hi = half * 128, (half + 1) * 128
            p_g = psg.tile([128, B], f32, tag="pg")
            for c in range(HC):
                nc.tensor.matmul(p_g[:], WihT[:, c, lo:hi], embT[:, c, :],
                                 start=(c == 0), stop=False)
            for c in range(HC):
                nc.tensor.matmul(p_g[:], WhhT[:, c, lo:hi], h0Tf[:, c, :],
                                 start=False, stop=False)
            nc.tensor.matmul(p_g[:], bsum[:, lo:hi], ones1[:],
                             start=False, stop=False)
            pg.append(p_g)

        # ---- all-gather ctxT shards -> full [512, 128] ----
        ctx_in = dram.tile([H, BL], f32, tag="ctx_in")
        nc.sync.dma_start(
            ctx_in[:].rearrange("(c p) j -> p c j", p=128), ctxT[:])
        ctx_all = dram.tile([NC * H, BL], f32, tag="ctx_all")
        nc.gpsimd.collective_compute(
            "AllGather", mybir.AluOpType.bypass,
            replica_groups=[list(range(NC))],
            ins=[ctx_in[:].opt()], outs=[ctx_all[:].opt()])
        ctxg = const.tile([128, HC, NC, BL], f32, tag="ctxg")
        ctx_all_v = ctx_all[:].rearrange("(k c p) j -> c p k j", p=128, c=HC)
        for c in range(HC):
            nc.sync.dma_start(ctxg[:, c, :, :], ctx_all_v[c])

        # ---- gates: ctx-dependent accumulation (after the gather) ----
        for half in range(2):
            lo, hi = half * 128, (half + 1) * 128
            for c in range(HC):
                nc.tensor.matmul(pg[half][:], WihT[:, HC + c, lo:hi],
                                 ctxg[:, c, :, :], start=False,
                                 stop=(c == HC - 1))

        # ---- LSTM cell elementwise (unit-shard, transposed [u, b]) ----
        # each gate in its own base-0 tile: DVE needs equal base partitions
        si = work.tile([UL, B], f32, tag="si")
        nc.scalar.activation(si[:], pg[0][0:UL, :], SIG)
        sf = work.tile([UL, B], f32, tag="sf")
        nc.scalar.activation(sf[:], pg[0][UL:128, :], SIG)
        tg = work.tile([UL, B], f32, tag="tg")
        nc.scalar.activation(tg[:], pg[1][0:UL, :], TANH)
        so = work.tile([UL, B], f32, tag="so")
        nc.scalar.activation(so[:], pg[1][UL:128, :], SIG)
        t1 = work.tile([UL, B], f32, tag="t1")
        nc.vector.tensor_mul(t1[:], sf[:], c0T[:])
        t2 = work.tile([UL, B], f32, tag="t2")
        nc.vector.tensor_mul(t2[:], si[:], tg[:])
        cnew = work.tile([UL, B], f32, tag="cnew")
        nc.vector.tensor_add(cnew[:], t1[:], t2[:])
        nc.sync.dma_start(cT_d[:], cnew[:])
        tanc = work.tile([UL, B], f32, tag="tanc")
        nc.scalar.activation(tanc[:], cnew[:], TANH)
        hnew = work.tile([UL, B], f32, tag="hnew")
        nc.vector.tensor_mul(hnew[:], so[:], tanc[:])
        nc.sync.dma_start(hT_d[:], hnew[:])

        # ---- all-gather h_new^T shards -> full [512, 128] ----
        h_in = dram.tile([UL, B], f32, tag="h_in")
        nc.sync.dma_start(h_in[:], hnew[:])
        h_all = dram.tile([H, B], f32, tag="h_all")
        nc.gpsimd.collective_compute(
            "AllGather", mybir.AluOpType.bypass,
            replica_groups=[list(range(NC))],
            ins=[h_in[:].opt()], outs=[h_all[:].opt()])
        hT = const.tile([128, HC, B], f32, tag="hT")
        h_all_v = h_all[:].rearrange("(c p) b -> c p b", p=128)
        for c in range(HC):
            nc.sync.dma_start(hT[:, c, :].bitcast(f32r), h_all_v[c].bitcast(f32r))

        # ---- fc: logits[b, v] = h_new @ fc_W.T + fc_b  (vocab shard) ----
        # bias prefill for the first 6 tiles overlaps the h gather
        NPRE = 6
        pf_tiles = {}
        for t in range(NPRE):
            pf = psum.tile([128, NT], f32, tag="mm")
            nc.tensor.matmul(pf[:], onesr[:].bitcast(f32r),
                             fcb[:, t * NT:(t + 1) * NT].bitcast(f32r),
                             start=True, stop=False)
            pf_tiles[t] = pf
        for t in range(VL // NT):
            fcw = fcw_tiles[t]
            if t < NPRE:
                pf = pf_tiles[t]
            else:
                pf = psum.tile([128, NT], f32, tag="mm")
                nc.tensor.matmul(pf[:], onesr[:].bitcast(f32r),
                                 fcb[:, t * NT:(t + 1) * NT].bitcast(f32r),
                                 start=True, stop=False)
            for c in range(HC):
                nc.tensor.matmul(pf[:], hT[:, c, :].bitcast(f32r),
                                 fcw[:, c, :].bitcast(f32r),
                                 start=False, stop=(c == HC - 1))
            lsb = outp.tile([128, NT], f32, tag="lsb")
            nc.vector.tensor_copy(lsb[:], pf[:])
            nc.sync.dma_start(logits_d[t], lsb[:])

    nc.compile()
    return nc


def _get_nc():
    if "nc" not in _CACHE:
        _CACHE["nc"] = _build()
    return _CACHE["nc"]


def _prep_in_maps(inputs):
    ids = np.asarray(inputs["input_ids"]).astype(np.int64)
    emb = np.asarray(inputs["emb"], dtype=np.float32)
    embT = np.ascontiguousarray(emb[ids].T)                    # [512, 128]
    h0 = np.asarray(inputs["h0"], dtype=np.float32)[0]         # [128, 512]
    h0T = np.ascontiguousarray(h0.T)                           # [512, 128]
    c0T = np.ascontiguousarray(np.asarray(inputs["c0"], dtype=np.float32)[0].T)
    enc = np.asarray(inputs["encoder_outputs"], dtype=np.float32)
    attn_W = np.asarray(inputs["attn_W"], dtype=np.float32)    # [512, 1024]
    WhT = np.ascontiguousarray(attn_W[:, :H].T)
    WeT = np.ascontiguousarray(attn_W[:, H:].T)
    attnb = np.asarray(inputs["attn_b"], dtype=np.float32).reshape(1, H)
    v4 = np.asarray(inputs["v"], dtype=np.float32).reshape(HC, 128)
    W_ih = np.asarray(inputs["W_ih"], dtype=np.float32)        # [2048, 1024]
    W_hh = np.asarray(inputs["W_hh"], dtype=np.float32)        # [2048, 512]
    b_ih = np.asarray(inputs["b_ih"], dtype=np.float32)
    b_hh = np.asarray(inputs["b_hh"], dtype=np.float32)
    fc_W = np.asarray(inputs["fc_W"], dtype=np.float32)        # [32000, 512]
    fc_b = np.asarray(inputs["fc_b"], dtype=np.float32)

    in_maps = []
    for k in range(NC):
        rows = np.concatenate([np.arange(g * H + k * UL, g * H + (k + 1) * UL)
                               for g in range(4)])             # i,f,g,o shard
        encT_k = np.ascontiguousarray(
            enc[k * BL:(k + 1) * BL].reshape(BS, H).T)         # [512, 1024]
        fcWT_k = np.ascontiguousarray(
            fc_W[k * VL:(k + 1) * VL].T.reshape(H, VL // NT, NT)
            .transpose(1, 0, 2))                               # [8, 512, 500]
        in_maps.append({
            "encT": encT_k,
            "WeT": WeT,
            "WhT": WhT,
            "h0Tl": np.ascontiguousarray(h0T[:, k * BL:(k + 1) * BL]),
            "h0Tf": h0T,
            "embT": embT,
            "attnb": attnb,
            "v4": v4,
            "c0T": np.ascontiguousarray(c0T[k * UL:(k + 1) * UL]),
            "WihT": np.ascontiguousarray(W_ih[rows].T),        # [1024, 256]
            "WhhT": np.ascontiguousarray(W_hh[rows].T),        # [512, 256]
            "bih": b_ih[rows].reshape(1, GW).copy(),
            "bhh": b_hh[rows].reshape(1, GW).copy(),
            "fcWT": fcWT_k,
            "fcb": fc_b[k * VL:(k + 1) * VL].reshape(1, VL).copy(),
        })
    return in_maps


def _assemble(results):
    pred = np.empty((B, V), np.float32)
    for k in range(NC):
        pred[:, k * VL:(k + 1) * VL] = (
            results[k]["logits"].transpose(1, 0, 2).reshape(B, VL))
    hT = np.concatenate([results[k]["hT"] for k in range(NC)], axis=0)
    cT = np.concatenate([results[k]["cT"] for k in range(NC)], axis=0)
    return pred, np.ascontiguousarray(hT.T)[None], np.ascontiguousarray(cT.T)[None]


def kernel(**inputs):
    from concourse.bass_utils import run_bass_kernel_spmd

    nc = _get_nc()
    in_maps = _prep_in_maps(inputs)
    res = run_bass_kernel_spmd(nc, in_maps, core_ids=list(range(NC)))
    return _assemble(res.results)


# revision 14
# speedup vs baseline: 1.2348x; 1.0371x over previous
"""DecoderRNN (attention + LSTM cell + vocab projection) on 8 TRN2 NeuronCores.

Sharding:
  - Attention: data-parallel over batch (16 rows/core), computed in a
    transposed [feature, batch*seq] layout so every matmul contracts over
    the partition dim.
  - LSTM cell: tensor-parallel over the 4H gate dim -- core k owns hidden
    units [64k, 64k+64) of each of i/f/g/o, for the full batch.  Needs the
    full rnn_in, so the per-core context shards are all-gathered first.
  - FC (vocab): tensor-parallel over V (4000 rows/core) after all-gathering
    the h_new shards.
Outputs per core: logits slab [8,128,500], hT/cT shards [64,128]; the host
reassembles the full (prediction, h_new, c_new).
"""

import sys

if "/opt/trn_rl_repo" not in sys.path:
    sys.path.insert(0, "/opt/trn_rl_repo")

import numpy as np

V, H, B, S = 32000, 512, 128, 64
NC = 8            # cores
BL = B // NC      # 16  local batch rows (attention shard)
UL = H // NC      # 64  local hidden units (lstm shard)
VL = V // NC      # 4000 local vocab rows (fc shard)
GW = 4 * UL       # 256 local gate rows (4 gates x 64 units)
NT = 500          # fc free-dim tile (8 tiles of 500 = 4000)
HC = H // 128     # 4   128-row chunks of H
DC = 2 * H // 128  # 8  128-row chunks of 2H (rnn_in)
BS = BL * S       # 1024 local (batch, seq) positions

_CACHE = {}


def _build():
    import concourse.bass as bass
    import concourse.bacc as bacc
    import concourse.mybir as mybir
    import concourse.tile as tile
    from contextlib import ExitStack

    f32 = mybir.dt.float32
    f32r = mybir.dt.float32r
    TANH = mybir.ActivationFunctionType.Tanh
    SIG = mybir.ActivationFunctionType.Sigmoid
    EXP = mybir.ActivationFunctionType.Exp
    ADD = mybir.AluOpType.add

    nc = bacc.Bacc("TRN2", target_bir_lowering=False, debug=False, num_devices=NC)

    def din(name, shape):
        return nc.dram_tensor(name, shape, f32, kind="ExternalInput").ap()

    def dout(name, shape):
        return nc.dram_tensor(name, shape, f32, kind="ExternalOutput").ap()

    encT_d = din("encT", [H, BS])          # enc shard, transposed [d, b*s]
    WeT_d = din("WeT", [H, H])             # attn_W[:,H:].T   [d, h]
    WhT_d = din("WhT", [H, H])             # attn_W[:,:H].T   [d, h]
    h0Tl_d = din("h0Tl", [H, BL])          # local h0.T slice [d, bl]
    h0Tf_d = din("h0Tf", [H, B])           # full h0.T (replicated)
    embT_d = din("embT", [H, B])           # full embedded.T (replicated)
    attnb_d = din("attnb", [1, H])
    v4_d = din("v4", [HC, 128])            # v reshaped [4,128]
    c0T_d = din("c0T", [UL, B])            # local c0.T slice (unit shard)
    WihT_d = din("WihT", [2 * H, GW])  # W_ih rows(shard).T  [1024, 256]
    WhhT_d = din("WhhT", [H, GW])      # W_hh rows(shard).T  [512, 256]
    bih_d = din("bih", [1, GW])
    bhh_d = din("bhh", [1, GW])
    fcWT_d = din("fcWT", [V // VL, H, NT])  # [8, 512, 500] per-tile contiguous
    fcb_d = din("fcb", [1, VL])

    logits_d = dout("logits", [VL // NT, B, NT])  # [8, 128, 500]
    hT_d = dout("hT", [UL, B])
    cT_d = dout("cT", [UL, B])

    # SEL[b, bs] = 1 if bs // S == b ; SEL[BL, :] = 1  (adds attn_b row)
    sel_np = np.zeros((BL + 1, BS), np.float32)
    for b in range(BL):
        sel_np[b, b * S:(b + 1) * S] = 1.0
    sel_np[BL, :] = 1.0
    sel_d = nc.inline_tensor(sel_np, name="sel").ap()
    ones_d = nc.inline_tensor(np.ones((1, 128), np.float32), name="onesc").ap()

    with tile.TileContext(nc) as tc, ExitStack() as ctx:
        const = ctx.enter_context(tc.tile_pool(name="const", bufs=1))
        work = ctx.enter_context(tc.tile_pool(name="work", bufs=2))
        fcwp = ctx.enter_context(tc.tile_pool(name="fcw", bufs=8))
        outp = ctx.enter_context(tc.tile_pool(name="outp", bufs=3))
        psum = ctx.enter_context(tc.tile_pool(name="psum", bufs=6, space="PSUM"))
        psg = ctx.enter_context(tc.tile_pool(name="psg", bufs=2, space="PSUM"))
        dram = ctx.enter_context(tc.tile_pool(name="dram", bufs=1, space="DRAM"))

        # ---- constant / weight loads (attention-critical first) ----
        h0Tl = const.tile([128, HC, BL], f32, tag="h0Tl")
        nc.sync.dma_start(h0Tl[:].bitcast(f32r),
                          h0Tl_d.rearrange("(c p) n -> p c n", p=128).bitcast(f32r))
        WhT = const.tile([128, HC, H], f32, tag="WhT")
        WhT_v = WhT_d.rearrange("(c p) n -> c p n", p=128)
        for c in range(HC):
            nc.sync.dma_start(WhT[:, c, :].bitcast(f32r), WhT_v[c].bitcast(f32r))
        WeT = const.tile([128, HC, H], f32, tag="WeT")
        WeT_v = WeT_d.rearrange("(c p) n -> c p n", p=128)
        for c in range(HC):
            nc.sync.dma_start(WeT[:, c, :].bitcast(f32r), WeT_v[c].bitcast(f32r))
        encT = const.tile([128, HC, BS], f32, tag="encT")
        encT_v = encT_d.rearrange("(c p) n -> c p n", p=128)
        for c in range(HC):
            nc.sync.dma_start(encT[:, c, :].bitcast(f32r), encT_v[c].bitcast(f32r))
        selt = const.tile([BL + 1, BS], f32, tag="selt")
        nc.sync.dma_start(selt[:].bitcast(f32r), sel_d[:].bitcast(f32r))
        hwb = const.tile([BL + 1, H], f32, tag="hwb")
        nc.sync.dma_start(hwb[BL:BL + 1, :].bitcast(f32r), attnb_d[:].bitcast(f32r))
        v4 = const.tile([128, HC], f32, tag="v4")
        nc.sync.dma_start(v4[:].bitcast(f32r), v4_d.rearrange("c p -> p c").bitcast(f32r))
        onesr = const.tile([1, 128], f32, tag="onesr")
        nc.sync.dma_start(onesr[:].bitcast(f32r), ones_d[:].bitcast(f32r))
        ones1 = const.tile([1, 128], f32, tag="ones1")
        nc.any.memset(ones1[:], 1.0)
        # lstm inputs (needed mid-kernel)
        WihT = const.tile([128, DC, GW], f32, tag="WihT")
        nc.sync.dma_start(WihT[:], WihT_d.rearrange("(c p) n -> p c n", p=128))
        WhhT = const.tile([128, HC, GW], f32, tag="WhhT")
        nc.sync.dma_start(WhhT[:], WhhT_d.rearrange("(c p) n -> p c n", p=128))
        h0Tf = const.tile([128, HC, B], f32, tag="h0Tf")
        nc.sync.dma_start(h0Tf[:], h0Tf_d.rearrange("(c p) n -> p c n", p=128))
        embT = const.tile([128, HC, B], f32, tag="embT")
        nc.sync.dma_start(embT[:], embT_d.rearrange("(c p) n -> p c n", p=128))
        bih = const.tile([1, GW], f32, tag="bih")
        nc.sync.dma_start(bih[:], bih_d[:])
        bhh = const.tile([1, GW], f32, tag="bhh")
        nc.sync.dma_start(bhh[:], bhh_d[:])
        c0T = const.tile([UL, B], f32, tag="c0T")
        nc.sync.dma_start(c0T[:], c0T_d[:])
        fcb = const.tile([1, VL], f32, tag="fcb")
        nc.sync.dma_start(fcb[:].bitcast(f32r), fcb_d[:].bitcast(f32r))

        # ---- prefetch all fc weight tiles (consumed in the fc loop) ----
        fcw_tiles = []
        for t in range(VL // NT):
            fcw = fcwp.tile([128, HC, NT], f32, tag="fcw")
            nc.sync.dma_start(
                fcw[:].bitcast(f32r),
                fcWT_d[t].rearrange("(c p) n -> p c n", p=128).bitcast(f32r))
            fcw_tiles.append(fcw)

        # ---- hW[b, h] = h0_loc @ W_h.T,   hwb = [hW ; attn_b] ----
        p_hw = psum.tile([BL, H], f32, tag="mm")
        for c in range(HC):
            nc.tensor.matmul(
                p_hw[:], h0Tl[:, c, :].bitcast(f32r), WhT[:, c, :].bitcast(f32r),
                start=(c == 0), stop=(c == HC - 1))
        nc.vector.tensor_copy(hwb[0:BL, :].bitcast(f32r), p_hw[:])

        # ---- energyT[h, bs] = tanh(W_e @ encT + hW[b] + attn_b) ----
        et = const.tile([128, HC, 2, 512], f32, tag="et")  # [h-chunk][n-tile]
        for m in range(HC):
            for n in range(2):
                pe = psum.tile([128, 512], f32, tag="mm")
                for c in range(HC):
                    nc.tensor.matmul(
                        pe[:],
                        WeT[:, c, m * 128:(m + 1) * 128].bitcast(f32r),
                        encT[:, c, n * 512:(n + 1) * 512].bitcast(f32r),
                        start=(c == 0), stop=False)
                nc.tensor.matmul(
                    pe[:],
                    hwb[:, m * 128:(m + 1) * 128].bitcast(f32r),
                    selt[:, n * 512:(n + 1) * 512].bitcast(f32r),
                    start=False, stop=True)
                nc.scalar.activation(et[:, m, n, :].bitcast(f32r), pe[:], TANH)

        # ---- scores[bs] = sum_h energyT[h, bs] * v[h]  (psum [1, 512] x2) ----
        scores = work.tile([1, BS], f32, tag="scores")
        for n in range(2):
            ps = psum.tile([1, 512], f32, tag="mm")
            for m in range(HC):
                nc.tensor.matmul(
                    ps[:], v4[:, m:m + 1].bitcast(f32r),
                    et[:, m, n, :].bitcast(f32r),
                    start=(m == 0), stop=(m == HC - 1))
            nc.vector.tensor_copy(scores[:, n * 512:(n + 1) * 512].bitcast(f32r), ps[:])

        # ---- softmax over s (no max-sub; scores are tanh-bounded) ----
        # broadcast scores across partitions with a K=1 ones matmul, then exp
        expbc = work.tile([128, BS], f32, tag="expbc")
        for n in range(2):
            pw = psum.tile([128, 512], f32, tag="mm")
            nc.tensor.matmul(
                pw[:], onesr[:].bitcast(f32r),
                scores[:, n * 512:(n + 1) * 512].bitcast(f32r),
                start=True, stop=True)
            nc.scalar.activation(expbc[:, n * 512:(n + 1) * 512], pw[:], EXP)
        sums = work.tile([128, BL], f32, tag="sums")
        nc.vector.tensor_reduce(
            sums[:], expbc[:].rearrange("p (g s) -> p g s", s=S),
            axis=mybir.AxisListType.X, op=ADD)
        recip = work.tile([128, BL], f32, tag="recip")
        nc.vector.reciprocal(recip[:], sums[:])

        # ---- ctxT[d, b] = (sum_s encT * exp) / sum_exp ----
        ctxT = const.tile([128, HC, BL], f32, tag="ctxT")
        for c in range(HC):
            prod = work.tile([128, BS], f32, tag="prod")
            nc.vector.tensor_mul(prod[:], encT[:, c, :], expbc[:])
            raw = work.tile([128, BL], f32, tag="raw")
            nc.vector.tensor_reduce(
                raw[:], prod[:].rearrange("p (g s) -> p g s", s=S),
                axis=mybir.AxisListType.X, op=ADD)
            nc.vector.tensor_mul(ctxT[:, c, :], raw[:], recip[:])

        # ---- gates pre-accumulation (emb, h0, bias): overlaps ctx gather ----
        bsum = work.tile([1, GW], f32, tag="bsum")
        nc.vector.tensor_add(bsum[:], bih[:], bhh[:])
        pg = []
        for half in range(2):
            lo, hi = half * 128, (half + 1) * 128
            p_g = psg.tile([128, B], f32, tag="pg")
            for c in range(HC):
                nc.tensor.matmul(p_g[:], WihT[:, c, lo:hi], embT[:, c, :],
                                 start=(c == 0), stop=False)
            for c in range(HC):
                nc.tensor.matmul(p_g[:], WhhT[:, c, lo:hi], h0Tf[:, c, :],
                                 start=False, stop=False)
            nc.tensor.matmul(p_g[:], bsum[:, lo:hi], ones1[:],
                             start=False, stop=False)
            pg.append(p_g)

        # ---- all-gather ctxT shards -> full [512, 128] ----
        ctx_in = dram.tile([H, BL], f32, tag="ctx_in")
        nc.sync.dma_start(
            ctx_in[:].rearrange("(c p) j -> p c j", p=128), ctxT[:])
        ctx_all = dram.tile([NC * H, BL], f32, tag="ctx_all")
        nc.gpsimd.collective_compute(
            "AllGather", mybir.AluOpType.bypass,
            replica_groups=[list(range(NC))],
            ins=[ctx_in[:].opt()], outs=[ctx_all[:].opt()])
        ctxg = const.tile([128, HC, NC, BL], f32, tag="ctxg")
        ctx_all_v = ctx_all[:].rearrange("(k c p) j -> c p k j", p=128, c=HC)
        for c in range(HC):
            nc.sync.dma_start(ctxg[:, c, :, :], ctx_all_v[c])

        # ---- gates: ctx-dependent accumulation (after the gather) ----
        for half in range(2):
            lo, hi = half * 128, (half + 1) * 128
            for c in range(HC):
                nc.tensor.matmul(pg[half][:], WihT[:, HC + c, lo:hi],
                                 ctxg[:, c, :, :], start=False,
                                 stop=(c == HC - 1))

        # ---- LSTM cell elementwise (unit-shard, transposed [u, b]) ----
        # each gate in its own base-0 tile: DVE needs equal base partitions
        si = work.tile([UL, B], f32, tag="si")
        nc.scalar.activation(si[:], pg[0][0:UL, :], SIG)
        sf = work.tile([UL, B], f32, tag="sf")
        nc.scalar.activation(sf[:], pg[0][UL:128, :], SIG)
        so = work.tile([UL, B], f32, tag="so")
        nc.scalar.activation(so[:], pg[1][UL:128, :], SIG)
        tg = work.tile([UL, B], f32, tag="tg")
        nc.scalar.activation(tg[:], pg[1][0:UL, :], TANH)
        t1 = work.tile([UL, B], f32, tag="t1")
        nc.vector.tensor_mul(t1[:], sf[:], c0T[:])
        t2 = work.tile([UL, B], f32, tag="t2")
        nc.vector.tensor_mul(t2[:], si[:], tg[:])
        cnew = work.tile([UL, B], f32, tag="cnew")
        nc.vector.tensor_add(cnew[:], t1[:], t2[:])
        tanc = work.tile([UL, B], f32, tag="tanc")
        nc.scalar.activation(tanc[:], cnew[:], TANH)
        hnew = work.tile([UL, B], f32, tag="hnew")
        nc.vector.tensor_mul(hnew[:], so[:], tanc[:])

        # ---- all-gather h_new^T shards -> full [512, 128] ----
        h_in = dram.tile([UL, B], f32, tag="h_in")
        nc.sync.dma_start(h_in[:], hnew[:])
        nc.sync.dma_start(hT_d[:], hnew[:])
        nc.sync.dma_start(cT_d[:], cnew[:])
        h_all = dram.tile([H, B], f32, tag="h_all")
        nc.gpsimd.collective_compute(
            "AllGather", mybir.AluOpType.bypass,
            replica_groups=[list(range(NC))],
            ins=[h_in[:].opt()], outs=[h_all[:].opt()])
        hT = const.tile([128, HC, B], f32, tag="hT")
        h_all_v = h_all[:].rearrange("(c p) b -> c p b", p=128)
        for c in range(HC):
            nc.sync.dma_start(hT[:, c, :].bitcast(f32r), h_all_v[c].bitcast(f32r))

        # ---- fc: logits[b, v] = h_new @ fc_W.T + fc_b  (vocab shard) ----
        # bias prefill for the first 6 tiles overlaps the h gather
        NPRE = 6
        pf_tiles = {}
        for t in range(NPRE):
            pf = psum.tile([128, NT], f32, tag="mm")
            nc.tensor.matmul(pf[:], onesr[:].bitcast(f32r),
                             fcb[:, t * NT:(t + 1) * NT].bitcast(f32r),
                             start=True, stop=False)
            pf_tiles[t] = pf
        for t in range(VL // NT):
            fcw = fcw_tiles[t]
            if t < NPRE:
                pf = pf_tiles[t]
            else:
                pf = psum.tile([128, NT], f32, tag="mm")
                nc.tensor.matmul(pf[:], onesr[:].bitcast(f32r),
                                 fcb[:, t * NT:(t + 1) * NT].bitcast(f32r),
                                 start=True, stop=False)
            for c in range(HC):
                nc.tensor.matmul(pf[:], hT[:, c, :].bitcast(f32r),
                                 fcw[:, c, :].bitcast(f32r),
                                 start=False, stop=(c == HC - 1))
            lsb = outp.tile([128, NT], f32, tag="lsb")
            nc.vector.tensor_copy(lsb[:], pf[:])
            nc.sync.dma_start(logits_d[t], lsb[:])

    nc.compile()
    return nc


def _get_nc():
    if "nc" not in _CACHE:
        _CACHE["nc"] = _build()
    return _CACHE["nc"]


def _prep_in_maps(inputs):
    ids = np.asarray(inputs["input_ids"]).astype(np.int64)
    emb = np.asarray(inputs["emb"], dtype=np.float32)
    embT = np.ascontiguousarray(emb[ids].T)                    # [512, 128]
    h0 = np.asarray(inputs["h0"], dtype=np.float32)[0]         # [128, 512]
    h0T = np.ascontiguousarray(h0.T)                           # [512, 128]
    c0T = np.ascontiguousarray(np.asarray(inputs["c0"], dtype=np.float32)[0].T)
    enc = np.asarray(inputs["encoder_outputs"], dtype=np.float32)
    attn_W = np.asarray(inputs["attn_W"], dtype=np.float32)    # [512, 1024]
    WhT = np.ascontiguousarray(attn_W[:, :H].T)
    WeT = np.ascontiguousarray(attn_W[:, H:].T)
    attnb = np.asarray(inputs["attn_b"], dtype=np.float32).reshape(1, H)
    v4 = np.asarray(inputs["v"], dtype=np.float32).reshape(HC, 128)
    W_ih = np.asarray(inputs["W_ih"], dtype=np.float32)        # [2048, 1024]
    W_hh = np.asarray(inputs["W_hh"], dtype=np.float32)        # [2048, 512]
    b_ih = np.asarray(inputs["b_ih"], dtype=np.float32)
    b_hh = np.asarray(inputs["b_hh"], dtype=np.float32)
    fc_W = np.asarray(inputs["fc_W"], dtype=np.float32)        # [32000, 512]
    fc_b = np.asarray(inputs["fc_b"], dtype=np.float32)

    in_maps = []
    for k in range(NC):
        rows = np.concatenate([np.arange(g * H + k * UL, g * H + (k + 1) * UL)
                               for g in range(4)])             # i,f,g,o shard
        encT_k = np.ascontiguousarray(
            enc[k * BL:(k + 1) * BL].reshape(BS, H).T)         # [512, 1024]
        fcWT_k = np.ascontiguousarray(
            fc_W[k * VL:(k + 1) * VL].T.reshape(H, VL // NT, NT)
            .transpose(1, 0, 2))                               # [8, 512, 500]
        in_maps.append({
            "encT": encT_k,
            "WeT": WeT,
            "WhT": WhT,
            "h0Tl": np.ascontiguousarray(h0T[:, k * BL:(k + 1) * BL]),
            "h0Tf": h0T,
            "embT": embT,
            "attnb": attnb,
            "v4": v4,
            "c0T": np.ascontiguousarray(c0T[k * UL:(k + 1) * UL]),
            "WihT": np.ascontiguousarray(W_ih[rows].T),        # [1024, 256]
            "WhhT": np.ascontiguousarray(W_hh[rows].T),        # [512, 256]
            "bih": b_ih[rows].reshape(1, GW).copy(),
            "bhh": b_hh[rows].reshape(1, GW).copy(),
            "fcWT": fcWT_k,
            "fcb": fc_b[k * VL:(k + 1) * VL].reshape(1, VL).copy(),
        })
    return in_maps


def _assemble(results):
    pred = np.empty((B, V), np.float32)
    for k in range(NC):
        pred[:, k * VL:(k + 1) * VL] = (
            results[k]["logits"].transpose(1, 0, 2).reshape(B, VL))
    hT = np.concatenate([results[k]["hT"] for k in range(NC)], axis=0)
    cT = np.concatenate([results[k]["cT"] for k in range(NC)], axis=0)
    return pred, np.ascontiguousarray(hT.T)[None], np.ascontiguousarray(cT.T)[None]


def kernel(**inputs):
    from concourse.bass_utils import run_bass_kernel_spmd

    nc = _get_nc()
    in_maps = _prep_in_maps(inputs)
    res = run_bass_kernel_spmd(nc, in_maps, core_ids=list(range(NC)))
    return _assemble(res.results)
